# revision 5
# baseline (speedup 1.0000x reference)
"""Trainium2 Bass kernel for the GCN discriminator — local_scatter routed.

Math (x:[N,1], b1=0): both GCN layers collapse to scalar message passing
y = D^-1/2 (A+I) D^-1/2 x. Per NC (dst shard of 12544 nodes), the 400k
per-edge random gathers are routed as: DVE expand (per-src broadcast) ->
GPSIMD local_scatter bucket-by-dst-row -> PE strided 128x128 transposes
-> local_scatter into per-dst-node segments -> DVE segmented reduce.
All engines stream; no random SBUF reads.
"""
import numpy as np
import concourse.bass as bass
import concourse.mybir as mybir
from concourse.tile import TileContext
from concourse import library_config

N_NODES = 100000
N_PAD = 100352
SRC_COLS = 784
SHARD = 12544
DST_PER_ROW = 98
N_GRAPHS = 64
NQ = 4
ROWS_PER_Q = 32
NCHUNKS_GRID = 98
F32 = mybir.dt.float32
F16 = mybir.dt.float16
I16 = mybir.dt.int16
AF = mybir.ActivationFunctionType
ALU = mybir.AluOpType
AX = mybir.AxisListType

WIN = 2046  # local_scatter max num_elems


# ---------------------------------------------------------------- host prep
def _even(v):
    return v + (v & 1)


def _build_structure(src, dst):
    """Routing schedule + per-NC int16 index arrays. See structure.py docs."""
    nc_of = dst // SHARD
    per_nc_raw = [(src[nc_of == c], dst[nc_of == c] - c * SHARD) for c in range(8)]

    deg_in_full = np.bincount(dst, minlength=N_PAD)
    rank_node = np.empty((8, 128, DST_PER_ROW), np.int64)
    Ksorted = np.empty((8, 128, DST_PER_ROW), np.int64)
    for c in range(8):
        degs = deg_in_full[c * SHARD:(c + 1) * SHARD].reshape(128, DST_PER_ROW)
        order = np.argsort(-degs, axis=1, kind="stable")
        rank_node[c] = order + np.arange(128)[:, None] * DST_PER_ROW
        Ksorted[c] = np.take_along_axis(degs, order, axis=1)
    prof_red = Ksorted.max(axis=(0, 1))
    offs_red = np.concatenate([[0], np.cumsum(prof_red)])
    G3_LEN = _even(int(offs_red[-1]))
    assert G3_LEN < 2 ** 15
    rank_of = np.empty((8, SHARD), np.int64)
    for c in range(8):
        flat = rank_node[c] + np.arange(128)[:, None] * 0  # ld values
        inv = np.empty(SHARD, np.int64)
        inv[flat.reshape(-1)] = np.tile(np.arange(DST_PER_ROW), 128)
        rank_of[c] = inv
    red_groups = []
    i = 0
    while i < DST_PER_ROW and prof_red[i] > 0:
        j = i
        while j < DST_PER_ROW and prof_red[j] == prof_red[i]:
            j += 1
        red_groups.append((int(prof_red[i]), i, j - i, int(offs_red[i])))
        i = j

    all_mult, all_cnt = [], []
    for c in range(8):
        s, ld = per_nc_raw[c]
        rs, cs = s // SRC_COLS, s % SRC_COLS
        rd = ld // DST_PER_ROW
        mult = np.zeros((128, NQ, SRC_COLS), np.int32)
        np.add.at(mult, (rs, rd // ROWS_PER_Q, cs), 1)
        cnt = np.zeros((128, 128), np.int32)
        np.add.at(cnt, (rs, rd), 1)
        all_mult.append(mult)
        all_cnt.append(cnt)
    B = int(max(cnt.max() for cnt in all_cnt))
    assert ROWS_PER_Q * B <= WIN

    teq_seg, exp_profiles = [], []
    for q in range(NQ):
        seg = max(int(np.count_nonzero(all_mult[c][:, q, :], axis=1).max())
                  for c in range(8))
        seg = _even(seg)
        prof = np.zeros(seg, np.int64)
        for c in range(8):
            mm = np.sort(all_mult[c][:, q, :], axis=1)[:, ::-1][:, :seg]
            prof = np.maximum(prof, mm.max(axis=0))
        teq_seg.append(seg)
        exp_profiles.append(prof)
    teq_off = np.concatenate([[0], np.cumsum(teq_seg)])

    exp_groups, exp_len = [], []
    for q in range(NQ):
        prof = exp_profiles[q]
        offs = np.concatenate([[0], np.cumsum(prof)])
        groups = []
        i = 0
        while i < len(prof) and prof[i] > 0:
            j = i
            while j < len(prof) and prof[j] == prof[i]:
                j += 1
            groups.append((int(prof[i]), i, j - i, int(offs[i])))
            i = j
        exp_groups.append(groups)
        exp_len.append(_even(int(offs[i])))

    n_win3 = -(-G3_LEN // WIN)
    win3 = [(w * WIN, min(WIN, G3_LEN - w * WIN)) for w in range(n_win3)]

    sched = dict(B=B, G3_LEN=G3_LEN, teq_seg=teq_seg, teq_off=teq_off,
                 exp_groups=exp_groups, exp_len=exp_len,
                 red_groups=red_groups, offs_red=offs_red, win3=win3)

    per_nc = []
    for c in range(8):
        s, ld = per_nc_raw[c]
        rs, cs = s // SRC_COLS, s % SRC_COLS
        rd = ld // DST_PER_ROW
        q = rd // ROWS_PER_Q
        mult = all_mult[c]

        idx0 = np.full((NQ, 128, SRC_COLS), -1, np.int16)
        slot_of_node = np.full((128, NQ, SRC_COLS), -1, np.int64)
        for q_ in range(NQ):
            mm = mult[:, q_, :]
            order = np.argsort(-mm, axis=1, kind="stable")
            nvalid = (np.take_along_axis(mm, order, axis=1) > 0).sum(axis=1)
            for r in range(128):
                nv = nvalid[r]
                idx0[q_, r, order[r, :nv]] = np.arange(nv, dtype=np.int16)
                slot_of_node[r, q_, order[r, :nv]] = np.arange(nv)

        okey = np.lexsort((cs, q, rs))
        rs_o, cs_o, q_o, rd_o, ld_o = rs[okey], cs[okey], q[okey], rd[okey], ld[okey]
        grp_change = np.concatenate([[True], (rs_o[1:] != rs_o[:-1]) |
                                     (q_o[1:] != q_o[:-1]) | (cs_o[1:] != cs_o[:-1])])
        gstart = np.flatnonzero(grp_change)
        glen = np.diff(np.concatenate([gstart, [len(okey)]]))
        copy_idx = np.arange(len(okey)) - np.repeat(gstart, glen)

        exp_pos = np.empty(len(okey), np.int64)
        for q_ in range(NQ):
            offs = np.concatenate([[0], np.cumsum(exp_profiles[q_])])
            mq = q_o == q_
            exp_pos[mq] = offs[slot_of_node[rs_o[mq], q_, cs_o[mq]]] + copy_idx[mq]

        bkey = np.lexsort((exp_pos, rd_o, rs_o))
        rs_b, rd_b = rs_o[bkey], rd_o[bkey]
        bchange = np.concatenate([[True], (rs_b[1:] != rs_b[:-1]) |
                                  (rd_b[1:] != rd_b[:-1])])
        bstart = np.flatnonzero(bchange)
        blen = np.diff(np.concatenate([bstart, [len(bkey)]]))
        j_b = np.arange(len(bkey)) - np.repeat(bstart, blen)
        j_of = np.empty(len(okey), np.int64)
        j_of[bkey] = j_b

        idx1 = [np.full((128, exp_len[q_]), -1, np.int16) for q_ in range(NQ)]
        for q_ in range(NQ):
            mq = q_o == q_
            idx1[q_][rs_o[mq], exp_pos[mq]] = \
                ((rd_o[mq] - ROWS_PER_Q * q_) * B + j_of[mq]).astype(np.int16)

        rank_e = rank_of[c][ld_o]
        nkey = np.lexsort((rs_o * B + j_of, ld_o))
        ld_n = ld_o[nkey]
        nchange = np.concatenate([[True], ld_n[1:] != ld_n[:-1]])
        nstart = np.flatnonzero(nchange)
        nlen = np.diff(np.concatenate([nstart, [len(nkey)]]))
        slot_n = np.arange(len(nkey)) - np.repeat(nstart, nlen)
        slot_of = np.empty(len(okey), np.int64)
        slot_of[nkey] = slot_n

        tgt = offs_red[rank_e] + slot_of
        idx3w = np.full((len(win3), 128, 128 * B), -1, np.int16)
        src_pos = rs_o * B + j_of
        for w, (w0, wlen) in enumerate(win3):
            m = (tgt >= w0) & (tgt < w0 + wlen)
            idx3w[w, rd_o[m], src_pos[m]] = (tgt[m] - w0).astype(np.int16)

        per_nc.append(dict(idx0=idx0, idx1=idx1, idx3w=idx3w,
                           rank_node=rank_node[c]))
    return sched, per_nc, deg_in_full


# ------------------------------------------------------------ bass builders
def _fix_walrus(nc):
    """Single sync-wait per extended instruction; move extras to NoOps."""
    ctr = 0
    for f in nc.m.functions:
        for b in f.blocks:
            newlist = []
            for ins in b.instructions:
                si = ins.sync_info
                if si is not None and si.on_wait and len(si.on_wait) > 1:
                    waits = list(si.on_wait)
                    for w in waits[1:]:
                        nop = mybir.InstNoOp(name=f"I-waitfix-{ctr}")
                        ctr += 1
                        nop.engine = ins.engine
                        nop.sync_info = mybir.SyncInfo(on_wait=[w], on_update=[])
                        nc.register_instruction(nop)
                        newlist.append(nop)
                    ins.sync_info = mybir.SyncInfo(on_wait=waits[:1],
                                                   on_update=list(si.on_update or []))
                newlist.append(ins)
            b.instructions[:] = newlist
    mybir.codegen_inst_isa_subclasses(nc)
    return nc


def _route_reduce(nc, pool, pspool, sched, idxt, streams, ident):
    """Run value streams through LS0/expand/LS1/transpose/LS3/reduce.

    streams: list of dicts with key 'tsrc' ([128, 784] F16 tile of per-node
    values). Returns list of ysum tiles [128, 98] F32 (segment sums).
    """
    B = sched["B"]
    teq_off = sched["teq_off"]
    TEQ = int(teq_off[-1])
    outs = []
    for si, st in enumerate(streams):
        tag = f"s{si}"
        teq = pool.tile([128, TEQ], F16, tag=f"teq{tag}")
        for q in range(NQ):
            seg = sched["teq_seg"][q]
            nc.gpsimd.local_scatter(
                teq[:, int(teq_off[q]):int(teq_off[q]) + seg],
                st["tsrc"][:], idxt["idx0"][:, q * SRC_COLS:(q + 1) * SRC_COLS],
                channels=128, num_elems=seg, num_idxs=SRC_COLS)
        G1 = pool.tile([128, 128 * B], F16, tag=f"G1{tag}")
        for q in range(NQ):
            el = sched["exp_len"][q]
            Eq = pool.tile([128, el], F16, tag=f"E{tag}{q}")
            ev = Eq[:]
            gend = sched["exp_groups"][q][-1]
            gend = gend[3] + gend[2] * gend[0]
            if gend < el:
                nc.vector.memset(ev[:, gend:el], 0.0)
            for (K, pos0, n, off) in sched["exp_groups"][q]:
                srcv = teq[:, int(teq_off[q]) + pos0:int(teq_off[q]) + pos0 + n]
                nc.vector.tensor_copy(
                    ev[:, off:off + n * K].rearrange("p (n k) -> p n k", n=n),
                    srcv.unsqueeze(2).broadcast_to((128, n, K)))
            nc.gpsimd.local_scatter(
                G1[:, ROWS_PER_Q * B * q:ROWS_PER_Q * B * (q + 1)],
                Eq[:], idxt["idx1"][q][:],
                channels=128, num_elems=ROWS_PER_Q * B, num_idxs=el)
        # transpose G1 (rd-major buckets) -> G2 (rs-major)
        G2 = pool.tile([128, 128 * B], F16, tag=f"G2{tag}")
        g1v = G1[:].rearrange("p (r j) -> p r j", j=B)
        g2v = G2[:].rearrange("p (r j) -> p r j", j=B)
        for j0 in range(0, B, 8):
            nb = min(8, B - j0)
            ps = pspool.tile([128, nb * 128], F16, tag="tp")
            for b in range(nb):
                nc.tensor.transpose(ps[:, b * 128:(b + 1) * 128],
                                    g1v[:, :, j0 + b], ident[:])
            nc.vector.tensor_copy(
                g2v[:, :, j0:j0 + nb],
                ps[:].rearrange("p (b r) -> p r b", b=nb))
        G3 = pool.tile([128, sched["G3_LEN"]], F16, tag=f"G3{tag}")
        for w, (w0, wlen) in enumerate(sched["win3"]):
            nc.gpsimd.local_scatter(
                G3[:, w0:w0 + wlen], G2[:],
                idxt["idx3"][:, w * 128 * B:(w + 1) * 128 * B],
                channels=128, num_elems=wlen, num_idxs=128 * B)
        reductions = [G3]
        if st.get("also_abs"):
            GA = pool.tile([128, sched["G3_LEN"]], F16, tag=f"GA{tag}")
            nc.scalar.activation(GA[:], G3[:], AF.Abs)
            reductions.append(GA)
        for ri, G in enumerate(reductions):
            ysum = pool.tile([128, DST_PER_ROW], F32, tag=f"ys{tag}{ri}")
            nc.vector.memset(ysum[:], 0.0)
            for (K, pos0, n, off) in sched["red_groups"]:
                nc.vector.tensor_reduce(
                    ysum[:, pos0:pos0 + n].unsqueeze(-1),
                    G[:, off:off + n * K].rearrange("p (n k) -> p n k", n=n),
                    axis=AX.X, op=ALU.add)
            outs.append(ysum)
    return outs


def _common_inputs(nc, sched):
    B = sched["B"]
    idx0_in = nc.dram_tensor("idx0", [128, NQ * SRC_COLS], I16, kind="ExternalInput")
    idx1_ins = [nc.dram_tensor(f"idx1_{q}", [128, sched["exp_len"][q]], I16,
                               kind="ExternalInput") for q in range(NQ)]
    idx3_in = nc.dram_tensor("idx3", [128, len(sched["win3"]) * 128 * B], I16,
                             kind="ExternalInput")
    ident_in = nc.dram_tensor("ident", [128, 128], F16, kind="ExternalInput")
    return idx0_in, idx1_ins, idx3_in, ident_in


def _load_common(nc, pool, sched, idx0_in, idx1_ins, idx3_in, ident_in):
    B = sched["B"]
    idx0 = pool.tile([128, NQ * SRC_COLS], I16, tag="idx0")
    nc.sync.dma_start(idx0[:], idx0_in.ap())
    idx1 = []
    for q in range(NQ):
        t = pool.tile([128, sched["exp_len"][q]], I16, tag=f"idx1_{q}")
        nc.sync.dma_start(t[:], idx1_ins[q].ap())
        idx1.append(t)
    idx3 = pool.tile([128, len(sched["win3"]) * 128 * B], I16, tag="idx3")
    nc.sync.dma_start(idx3[:], idx3_in.ap())
    ident = pool.tile([128, 128], F16, tag="ident")
    nc.sync.dma_start(ident[:], ident_in.ap())
    return dict(idx0=idx0, idx1=idx1, idx3=idx3), ident


def build_launch1(sched):
    nc = bass.Bass("TRN2", target_bir_lowering=False)
    x_in = nc.dram_tensor("x784", [128, SRC_COLS], F32, kind="ExternalInput")
    deg_in = nc.dram_tensor("deg784", [128, SRC_COLS], F32, kind="ExternalInput")
    xr_in = nc.dram_tensor("x_rank", [128, DST_PER_ROW], F32, kind="ExternalInput")
    degr_in = nc.dram_tensor("deg_rank", [128, DST_PER_ROW], F32, kind="ExternalInput")
    idx0_in, idx1_ins, idx3_in, ident_in = _common_inputs(nc, sched)
    y_out = nc.dram_tensor("y_out", [128, DST_PER_ROW], F32, kind="ExternalOutput")

    with TileContext(nc) as tc:
        nc.gpsimd.load_library(library_config.local_scatter)
        with tc.tile_pool(name="c", bufs=1) as pool, \
             tc.tile_pool(name="ps", bufs=2, space="PSUM") as pspool:
            idxt, ident = _load_common(nc, pool, sched, idx0_in, idx1_ins,
                                       idx3_in, ident_in)
            xs = pool.tile([128, SRC_COLS], F32)
            ds = pool.tile([128, SRC_COLS], F32)
            nc.sync.dma_start(xs[:], x_in.ap())
            nc.sync.dma_start(ds[:], deg_in.ap())
            sq = pool.tile([128, SRC_COLS], F32)
            nc.scalar.activation(sq[:], ds[:], AF.Sqrt)
            dinv = pool.tile([128, SRC_COLS], F32)
            nc.vector.reciprocal(dinv[:], sq[:])
            tsf = pool.tile([128, SRC_COLS], F32)
            nc.vector.tensor_mul(tsf[:], dinv[:], xs[:])
            tsrc = pool.tile([128, SRC_COLS], F16)
            nc.vector.tensor_copy(tsrc[:], tsf[:])

            (ysum,) = _route_reduce(nc, pool, pspool, sched, idxt,
                                    [dict(tsrc=tsrc)], ident)

            xr = pool.tile([128, DST_PER_ROW], F32)
            dr = pool.tile([128, DST_PER_ROW], F32)
            nc.sync.dma_start(xr[:], xr_in.ap())
            nc.sync.dma_start(dr[:], degr_in.ap())
            sqr = pool.tile([128, DST_PER_ROW], F32)
            nc.scalar.activation(sqr[:], dr[:], AF.Sqrt)
            dinvr = pool.tile([128, DST_PER_ROW], F32)
            nc.vector.reciprocal(dinvr[:], sqr[:])
            t1 = pool.tile([128, DST_PER_ROW], F32)
            nc.vector.tensor_mul(t1[:], dinvr[:], xr[:])       # x*dinv
            nc.vector.tensor_mul(t1[:], t1[:], dinvr[:])       # x*dinv^2
            nc.vector.tensor_add(t1[:], t1[:], ysum[:])        # + S ... wait
            # y = dinv*(S + dinv^2 * x)?  y = dinv*S + dinv^2*x*dinv:
            # S is sum of t=dinv_s*x_s; y = dinv_d*S + dinv_d^2*x_d*dinv_d.
            # t1 currently = x*dinv^2 + S; multiplying by dinv gives
            # dinv*S + x*dinv^3. Correct.
            y = pool.tile([128, DST_PER_ROW], F32)
            nc.vector.tensor_mul(y[:], dinvr[:], t1[:])
            nc.sync.dma_start(y_out.ap(), y[:])
    return _fix_walrus(nc)


def build_launch2(sched):
    nc = bass.Bass("TRN2", target_bir_lowering=False)
    y_in = nc.dram_tensor("y784", [128, SRC_COLS], F32, kind="ExternalInput")
    deg_in = nc.dram_tensor("deg784", [128, SRC_COLS], F32, kind="ExternalInput")
    yr_in = nc.dram_tensor("y_rank", [128, DST_PER_ROW], F32, kind="ExternalInput")
    degr_in = nc.dram_tensor("deg_rank", [128, DST_PER_ROW], F32, kind="ExternalInput")
    idx0_in, idx1_ins, idx3_in, ident_in = _common_inputs(nc, sched)
    oh_in = nc.dram_tensor("pool_oh", [128, NCHUNKS_GRID * 64], F16,
                           kind="ExternalInput")
    uvb128_in = nc.dram_tensor("uvb128", [128, 96], F32, kind="ExternalInput")
    pool_out = nc.dram_tensor("pool_out", [64, 32], F32, kind="ExternalOutput")

    with TileContext(nc) as tc:
        nc.gpsimd.load_library(library_config.local_scatter)
        with tc.tile_pool(name="c", bufs=1) as pool, \
             tc.tile_pool(name="ps", bufs=2, space="PSUM") as pspool:
            idxt, ident = _load_common(nc, pool, sched, idx0_in, idx1_ins,
                                       idx3_in, ident_in)
            ys = pool.tile([128, SRC_COLS], F32)
            ds = pool.tile([128, SRC_COLS], F32)
            nc.sync.dma_start(ys[:], y_in.ap())
            nc.sync.dma_start(ds[:], deg_in.ap())
            sq = pool.tile([128, SRC_COLS], F32)
            nc.scalar.activation(sq[:], ds[:], AF.Sqrt)
            dinv = pool.tile([128, SRC_COLS], F32)
            nc.vector.reciprocal(dinv[:], sq[:])
            st = pool.tile([128, SRC_COLS], F32)
            nc.vector.tensor_mul(st[:], ys[:], dinv[:])
            ts16 = pool.tile([128, SRC_COLS], F16)
            nc.vector.tensor_copy(ts16[:], st[:])

            # one signed stream; Sp/Sn recovered from signed + abs reduces
            S1, S2 = _route_reduce(nc, pool, pspool, sched, idxt,
                                   [dict(tsrc=ts16, also_abs=True)], ident)

            yr = pool.tile([128, DST_PER_ROW], F32)
            dr = pool.tile([128, DST_PER_ROW], F32)
            nc.sync.dma_start(yr[:], yr_in.ap())
            nc.sync.dma_start(dr[:], degr_in.ap())
            sqr = pool.tile([128, DST_PER_ROW], F32)
            nc.scalar.activation(sqr[:], dr[:], AF.Sqrt)
            dinvr = pool.tile([128, DST_PER_ROW], F32)
            nc.vector.reciprocal(dinvr[:], sqr[:])
            P = pool.tile([128, DST_PER_ROW], F32)
            Q = pool.tile([128, DST_PER_ROW], F32)
            for gi, (out, sgn, scale) in enumerate(((P, 1.0, 1.0), (Q, -1.0, -1.0))):
                t2 = pool.tile([128, DST_PER_ROW], F32, tag=f"t2{gi}")
                # Sp = (S2 + S1)/2 ; Sn = (S2 - S1)/2
                s12 = pool.tile([128, DST_PER_ROW], F32, tag=f"s12{gi}")
                if sgn > 0:
                    nc.vector.tensor_add(s12[:], S2[:], S1[:])
                else:
                    nc.vector.tensor_sub(s12[:], S2[:], S1[:])
                nc.vector.tensor_scalar_mul(s12[:], s12[:], 0.5)
                nc.scalar.activation(t2[:], yr[:], AF.Relu, scale=scale)
                nc.vector.tensor_mul(t2[:], t2[:], dinvr[:])
                nc.vector.tensor_mul(t2[:], t2[:], dinvr[:])
                nc.vector.tensor_add(t2[:], t2[:], s12[:])
                nc.vector.tensor_mul(out[:], t2[:], dinvr[:])

            # phase C: z = relu(P (x) u + Q (x) v + b2) on DVE, pool on PE
            uvb128 = pool.tile([128, 96], F32)
            nc.sync.dma_start(uvb128[:], uvb128_in.ap())
            oh = pool.tile([128, NCHUNKS_GRID * 64], F16)
            nc.sync.dma_start(oh[:], oh_in.ap())
            NZ = NCHUNKS_GRID * 32
            zsum = pool.tile([128, NZ], F32)
            zv = zsum[:].rearrange("p (j f) -> p j f", f=32)
            ub = uvb128[:, 0:32].unsqueeze(1).broadcast_to((128, NCHUNKS_GRID, 32))
            vb = uvb128[:, 32:64].unsqueeze(1).broadcast_to((128, NCHUNKS_GRID, 32))
            bb = uvb128[:, 64:96].unsqueeze(1).broadcast_to((128, NCHUNKS_GRID, 32))
            Pbc = P[:].unsqueeze(2).broadcast_to((128, DST_PER_ROW, 32))
            Qbc = Q[:].unsqueeze(2).broadcast_to((128, DST_PER_ROW, 32))
            ztmp = pool.tile([128, NZ], F32)
            zvt = ztmp[:].rearrange("p (j f) -> p j f", f=32)
            nc.vector.tensor_tensor(zv, Pbc, ub, op=ALU.mult)
            nc.vector.tensor_tensor(zvt, Qbc, vb, op=ALU.mult)
            nc.vector.tensor_add(zsum[:], zsum[:], ztmp[:])
            nc.vector.tensor_tensor(zv, zv, bb, op=ALU.add)
            h2 = pool.tile([128, NZ], F16)
            nc.scalar.activation(h2[:], zsum[:], AF.Relu)
            h2v = h2[:].rearrange("p (j f) -> p j f", f=32)
            pool_ps = pspool.tile([64, 32], F32, tag="pool")
            for ci in range(NCHUNKS_GRID):
                nc.tensor.matmul(pool_ps[:], oh[:, ci * 64:(ci + 1) * 64],
                                 h2v[:, ci, :], start=(ci == 0),
                                 stop=(ci == NCHUNKS_GRID - 1))
            pooled = pool.tile([64, 32], F32)
            nc.vector.tensor_copy(pooled[:], pool_ps[:])
            nc.sync.dma_start(pool_out.ap(), pooled[:])
    return _fix_walrus(nc)


def build_launch3():
    nc = bass.Bass("TRN2", target_bir_lowering=False)
    parts_in = nc.dram_tensor("partials", [64, 8 * 32], F32, kind="ExternalInput")
    cnt_in = nc.dram_tensor("cnt", [64, 1], F32, kind="ExternalInput")
    wfc_in = nc.dram_tensor("wfc_row", [64, 32], F32, kind="ExternalInput")
    bfc_in = nc.dram_tensor("bfc", [64, 1], F32, kind="ExternalInput")
    out = nc.dram_tensor("out", [64, 1], F32, kind="ExternalOutput")
    with TileContext(nc) as tc:
        with tc.tile_pool(name="p", bufs=1) as pool:
            ps = pool.tile([64, 8 * 32], F32)
            nc.sync.dma_start(ps[:], parts_in.ap())
            acc = pool.tile([64, 32], F32)
            nc.vector.tensor_copy(acc[:], ps[:, 0:32])
            for c in range(1, 8):
                nc.vector.tensor_add(acc[:], acc[:], ps[:, 32 * c:32 * (c + 1)])
            cnt = pool.tile([64, 1], F32)
            nc.sync.dma_start(cnt[:], cnt_in.ap())
            cmax = pool.tile([64, 1], F32)
            nc.vector.tensor_scalar_max(cmax[:], cnt[:], 1.0)
            cinv = pool.tile([64, 1], F32)
            nc.vector.reciprocal(cinv[:], cmax[:])
            nc.vector.tensor_scalar_mul(acc[:], acc[:], cinv[:])
            wfc = pool.tile([64, 32], F32)
            nc.sync.dma_start(wfc[:], wfc_in.ap())
            nc.vector.tensor_mul(acc[:], acc[:], wfc[:])
            dot = pool.tile([64, 1], F32)
            nc.vector.tensor_reduce(dot[:], acc[:], axis=AX.X, op=ALU.add)
            bfc = pool.tile([64, 1], F32)
            nc.sync.dma_start(bfc[:], bfc_in.ap())
            nc.vector.tensor_add(dot[:], dot[:], bfc[:])
            res = pool.tile([64, 1], F32)
            nc.scalar.activation(res[:], dot[:], AF.Sigmoid)
            nc.sync.dma_start(out.ap(), res[:])
    return _fix_walrus(nc)


# ------------------------------------------------------------------ runner
_RUNNERS = {}


def _make_runner(key, nc, n_cores):
    """jit-compiled SPMD runner with device-resident input support."""
    import jax
    from jax.sharding import Mesh, PartitionSpec
    from jax.experimental.shard_map import shard_map
    from concourse.bass2jax import (_bass_exec_p, install_neuronx_cc_hook,
                                    partition_id_tensor)
    install_neuronx_cc_hook()
    partition_name = nc.partition_id_tensor.name if nc.partition_id_tensor else None
    in_names, out_names, out_avals, zero_outs = [], [], [], []
    for alloc in nc.m.functions[0].allocations:
        if not isinstance(alloc, mybir.MemoryLocationSet):
            continue
        name = alloc.memorylocations[0].name
        if alloc.kind == "ExternalInput":
            if name != partition_name:
                in_names.append(name)
        elif alloc.kind == "ExternalOutput":
            shape = tuple(alloc.tensor_shape)
            dtype = mybir.dt.np(alloc.dtype)
            out_names.append(name)
            out_avals.append(jax.core.ShapedArray(shape, dtype))
            zero_outs.append(np.zeros(shape, dtype))
    n_params, n_outs = len(in_names), len(out_avals)
    in_names_all = in_names + out_names + ([partition_name] if partition_name else [])

    def _body(*args):
        operands = list(args)
        if partition_name is not None:
            operands.append(partition_id_tensor())
        return tuple(_bass_exec_p.bind(
            *operands, out_avals=tuple(out_avals), in_names=tuple(in_names_all),
            out_names=tuple(out_names), lowering_input_output_aliases=(),
            sim_require_finite=False, sim_require_nnan=False, nc=nc))

    import jax as _jax
    devices = _jax.devices()[:n_cores]
    mesh = Mesh(np.asarray(devices), ("core",))
    sharded = _jax.jit(
        shard_map(_body, mesh=mesh,
                  in_specs=(PartitionSpec("core"),) * (n_params + n_outs),
                  out_specs=(PartitionSpec("core"),) * n_outs, check_rep=False),
        keep_unused=True)

    def run(in_maps, timing_iters=0):
        import time
        concat_in = [np.concatenate([np.asarray(in_maps[c][n]) for c in range(n_cores)],
                                    axis=0) for n in in_names]
        concat_zeros = [np.zeros((n_cores * z.shape[0], *z.shape[1:]), z.dtype)
                        for z in zero_outs]
        out_arrs = sharded(*concat_in, *concat_zeros)
        _jax.block_until_ready(out_arrs)
        dt = None
        if timing_iters:
            sharding = _jax.sharding.NamedSharding(mesh, PartitionSpec("core"))
            dev_in = [_jax.device_put(a, sharding) for a in concat_in]
            dev_zero = [_jax.device_put(a, sharding) for a in concat_zeros]
            iter_ts = []
            for _ in range(timing_iters):
                t0 = time.perf_counter()
                out_arrs2 = sharded(*dev_in, *dev_zero)
                _jax.block_until_ready(out_arrs2)
                iter_ts.append(time.perf_counter() - t0)
            dt = min(iter_ts)
        return [{n: np.asarray(out_arrs[i]).reshape(n_cores, *out_avals[i].shape)[c]
                 for i, n in enumerate(out_names)} for c in range(n_cores)], dt
    return run


# ------------------------------------------------------------------- entry
def kernel(x, edge_index, batch, W1, b1, W2, b2, Wfc, bfc, _timing=None):
    assert np.all(np.asarray(b1) == 0.0), "kernel exploits b1 == 0"
    x = np.asarray(x, np.float32)[:, 0]
    ei = np.asarray(edge_index, np.int64)
    batch_np = np.asarray(batch, np.int64)
    src, dst = ei[0], ei[1]

    sched, per_nc, deg_in_full = _build_structure(src, dst)
    deg_f = (deg_in_full + 1).astype(np.float32)
    x_ext = np.zeros(N_PAD, np.float32)
    x_ext[:N_NODES] = x

    w = np.asarray(W1, np.float32)[0]
    u = np.maximum(w, 0.0) @ np.asarray(W2, np.float32)
    v = np.maximum(-w, 0.0) @ np.asarray(W2, np.float32)
    uvb128 = np.tile(np.concatenate([u, v, np.asarray(b2, np.float32)]).astype(np.float32)[None, :], (128, 1))

    x784 = x_ext.reshape(128, SRC_COLS)
    deg784 = deg_f.reshape(128, SRC_COLS)
    ident = np.eye(128, dtype=np.float16)

    def common_inputs(p):
        return {
            "idx0": p["idx0"].transpose(1, 0, 2).reshape(128, NQ * SRC_COLS),
            **{f"idx1_{q}": p["idx1"][q] for q in range(NQ)},
            "idx3": p["idx3w"].transpose(1, 0, 2).reshape(128, -1),
            "ident": ident,
        }

    in_maps1 = []
    for c in range(8):
        p = per_nc[c]
        node = c * SHARD + p["rank_node"]          # [128, 98] global ids
        in_maps1.append({
            "x784": x784, "deg784": deg784,
            "x_rank": x_ext[node], "deg_rank": deg_f[node],
            **common_inputs(p),
        })
    if "L1" not in _RUNNERS:
        _RUNNERS["L1"] = _make_runner("L1", build_launch1(sched), 8)
    res1, dt1 = _RUNNERS["L1"](in_maps1, timing_iters=(_timing or 0))

    y_ext = np.zeros(N_PAD, np.float32)
    for c in range(8):
        node = c * SHARD + per_nc[c]["rank_node"]
        y_ext[node.reshape(-1)] = res1[c]["y_out"].reshape(-1)
    y784 = y_ext.reshape(128, SRC_COLS)

    in_maps2 = []
    for c in range(8):
        p = per_nc[c]
        node = c * SHARD + p["rank_node"]
        # pooling one-hot: ordinal o = rd*98 + rank -> node[rd, rank]
        oh = np.zeros((128, NCHUNKS_GRID * 64), np.float32)
        o = np.arange(SHARD)
        nid = node.reshape(-1)[o]
        real = nid < N_NODES
        g = np.where(real, batch_np[np.minimum(nid, N_NODES - 1)], 0)
        ci, pi = o // 128, o % 128
        oh[pi[real], ci[real] * 64 + g[real]] = 1.0
        in_maps2.append({
            "y784": y784, "deg784": deg784,
            "y_rank": y_ext[node], "deg_rank": deg_f[node],
            **common_inputs(p),
            "pool_oh": oh.astype(np.float16),
            "uvb128": uvb128,
        })
    if "L2" not in _RUNNERS:
        _RUNNERS["L2"] = _make_runner("L2", build_launch2(sched), 8)
    res2, dt2 = _RUNNERS["L2"](in_maps2, timing_iters=(_timing or 0))

    partials = np.stack([res2[c]["pool_out"] for c in range(8)])
    parts_in = partials.transpose(1, 0, 2).reshape(64, 8 * 32).astype(np.float32)
    cnt = np.bincount(batch_np, minlength=64).astype(np.float32).reshape(64, 1)
    wfc_row = np.tile(np.asarray(Wfc, np.float32).reshape(1, 32), (64, 1))
    bfc_col = np.full((64, 1), np.asarray(bfc, np.float32).reshape(()), np.float32)
    in3 = {"partials": parts_in, "cnt": cnt, "wfc_row": wfc_row, "bfc": bfc_col}
    if "L3" not in _RUNNERS:
        _RUNNERS["L3"] = _make_runner("L3", build_launch3(), 8)
    res3, dt3 = _RUNNERS["L3"]([in3] * 8, timing_iters=(_timing or 0))
    if _timing is not None:
        kernel._last_times = (dt1, dt2, dt3)
    return res3[0]["out"].astype(np.float32)
